# revision 7
# baseline (speedup 1.0000x reference)
"""AceStep GQA attention block on 8 Trainium2 NeuronCores.

Sharding: DP over batch (B=2) x sequence-parallel over S within each batch
group (4 cores each own 512 query positions).  Each core computes K/V for its
own 512 positions, AllGathers K^T/V (bf16) within its 4-core group, then runs
full attention for its query slice and the complete output projection row
block.  No output collective needed - each core owns a distinct slice.

v2 restructure (vs v1 baseline at ~725us):
  - all matmuls bf16 (weights pre-cast + pre-tiled on host, one DMA each)
  - K/V projections first, single AllGather issued early; Q projection and
    its norm/rope overlap the collective in flight
  - rmsnorm+rope fused: tensor_tensor_reduce (square+sum in one DVE op) +
    scalar_tensor_tensor ((psum*scale)*rope_coeff in one DVE op)
  - softmax denominator off the tensor engine: DVE tree-adds of exp tiles +
    one ones-matmul per head for the partition-sum broadcast; 1/den via
    DVE reciprocal_approx_fast.  Scalar engine runs only Exp during
    attention (no activation-table thrash).
  - exp batched per head-pair ([128,1024] per s-tile), p-tile buffer
    double-buffered so ACT never waits on the AV matmuls.

Device layouts (per core, c = 4*b + j):
  hsP   [128, it 16, t 512]  hidden[b].T tile-major (partition = i % 128)
  wkP/wvP [128, it 16, n 512]   W^T tile-major
  wqP/woP [oc 4][128, it/kk 16, n 512]
  cw*/sw* [128, tt 4, d 128]  rope coeffs (norm weight + scale folded in)
  outT  [HS, SC] f32
"""

import numpy as np

H, KV, D = 16, 4, 128
HD = D // 2
B, S, HS = 2, 2048, 2048
EPS = 1e-6
NCORES = 8
TPG = 4              # cores per batch group (sequence split)
SC = S // TPG        # 512 sequence positions per core
TT = SC // 128       # 4 t-tiles per core
IT = HS // 128       # 16 contraction tiles
ST = S // 128        # 16 s-tiles (full sequence)
OC = 4               # 512-wide output chunks for Q/O projections
GROUPS = [[0, 1, 2, 3], [4, 5, 6, 7]]

_BUILT = {}


def _build_program():
    from contextlib import ExitStack

    import concourse.bass as bass
    import concourse.bacc as bacc
    import concourse.mybir as mybir
    import concourse.tile as tile
    from concourse.masks import make_identity

    f32 = mybir.dt.float32
    bf16 = mybir.dt.bfloat16
    AF = mybir.ActivationFunctionType
    ALU = mybir.AluOpType

    nc = bacc.Bacc("TRN2", target_bir_lowering=False, debug=False,
                   num_devices=NCORES)

    # ---- external I/O (per core) ----
    hsP = nc.dram_tensor("hsP", [128, IT * SC], bf16, kind="ExternalInput").ap()
    wkP = nc.dram_tensor("wkP", [128, IT * 512], bf16, kind="ExternalInput").ap()
    wvP = nc.dram_tensor("wvP", [128, IT * 512], bf16, kind="ExternalInput").ap()
    wqP = nc.dram_tensor("wqP", [OC, 128, IT * 512], bf16,
                         kind="ExternalInput").ap()
    woP = nc.dram_tensor("woP", [OC, 128, IT * 512], bf16,
                         kind="ExternalInput").ap()
    cwq = nc.dram_tensor("cwq", [128, TT * D], f32, kind="ExternalInput").ap()
    swq = nc.dram_tensor("swq", [128, TT * D], f32, kind="ExternalInput").ap()
    cwk = nc.dram_tensor("cwk", [128, TT * D], f32, kind="ExternalInput").ap()
    swk = nc.dram_tensor("swk", [128, TT * D], f32, kind="ExternalInput").ap()
    outT = nc.dram_tensor("outT", [HS, SC], f32, kind="ExternalOutput").ap()

    tc_cm = tile.TileContext(nc)
    ctx = ExitStack()
    tc = tc_cm.__enter__()
    try:
        ep = ctx.enter_context
        const_pool = ep(tc.tile_pool(name="const", bufs=1))
        w_pool = ep(tc.tile_pool(name="w", bufs=2))
        scr_pool = ep(tc.tile_pool(name="scr", bufs=2))
        rope_pool = ep(tc.tile_pool(name="rope", bufs=2))
        qT_pool = ep(tc.tile_pool(name="qT", bufs=1))
        kv_pool = ep(tc.tile_pool(name="kv", bufs=1))
        osb_pool = ep(tc.tile_pool(name="osb", bufs=3))
        dram_pool = ep(tc.tile_pool(name="dram", bufs=1, space="DRAM"))

        # scoped pools released before the attention phase needs the space
        # (pool releases must be LIFO: kvloc closes first, then hs)
        ctx_hs = ExitStack()      # hs + projection psum (until Q proj done)
        ctx_kv = ExitStack()      # local kT/v staging (until pack DMAs done)
        hs_pool = ctx_hs.enter_context(tc.tile_pool(name="hs", bufs=1))
        mm_ps = ctx_hs.enter_context(
            tc.tile_pool(name="mm_ps", bufs=3, space="PSUM"))
        tr_ps = ctx_hs.enter_context(
            tc.tile_pool(name="tr_ps", bufs=2, space="PSUM"))
        kvloc_pool = ctx_kv.enter_context(tc.tile_pool(name="kvloc", bufs=1))

        # ---- constants ----
        ident = const_pool.tile([128, 128], bf16)
        make_identity(nc, ident)
        ones_bf = const_pool.tile([128, 128], bf16)
        nc.vector.memset(ones_bf, 1.0)
        eps_sb = const_pool.tile([128, 1], f32)
        nc.vector.memset(eps_sb, EPS)

        cwq_sb = const_pool.tile([128, TT * D], f32)
        swq_sb = const_pool.tile([128, TT * D], f32)
        cwk_sb = const_pool.tile([128, TT * D], f32)
        swk_sb = const_pool.tile([128, TT * D], f32)
        for (dst, src) in ((cwq_sb, cwq), (swq_sb, swq),
                           (cwk_sb, cwk), (swk_sb, swk)):
            nc.sync.dma_start(out=dst[:], in_=src)

        hs_sb = hs_pool.tile([128, IT * SC], bf16)
        nc.sync.dma_start(out=hs_sb[:], in_=hsP)

        def hs_tile(it, tt):
            # stationary [128 i, 128 t]
            off = it * SC + tt * 128
            return hs_sb[:, off: off + 128]

        def norm_rope(ps, nh, cw_t, sw_t, dst):
            """ps: psum [128 t, nh*D]; dst: bf16 AP [128, nh*D].
            scale = rsqrt(mean(x^2)+eps) via exp(-0.5*ln(sum/D+eps)) on ACT;
            square+sum fused on DVE; (x*scale)*rope fused on DVE."""
            x_sb = scr_pool.tile([128, nh * D], f32, tag="xsb", name="x_sb")
            nc.vector.tensor_copy(x_sb[:], ps[:])
            ssum = scr_pool.tile([128, nh], f32, tag="ssum", name="ssum")
            sq = scr_pool.tile([128, nh * D], f32, tag="sq", name="sq")
            nc.vector.tensor_mul(sq[:], x_sb[:], x_sb[:])
            nc.vector.reduce_sum(
                ssum[:], sq.rearrange("p (h d) -> p h d", d=D),
                axis=mybir.AxisListType.X)
            lnm = scr_pool.tile([128, nh], f32, tag="lnm", name="lnm")
            nc.scalar.activation(lnm[:], ssum[:], AF.Ln, bias=eps_sb[:],
                                 scale=1.0 / D)
            sc_t = scr_pool.tile([128, nh], f32, tag="sct", name="sc_t")
            nc.scalar.activation(sc_t[:], lnm[:], AF.Exp, scale=-0.5)
            for hh in range(nh):
                t1 = scr_pool.tile([128, D], f32, tag="t1", name="t1")
                t2 = scr_pool.tile([128, D], f32, tag="t2", name="t2")
                blk = x_sb[:, hh * D:(hh + 1) * D]
                s_hh = sc_t[:, hh:hh + 1]
                nc.vector.scalar_tensor_tensor(
                    t1[:], blk, s_hh, cw_t, op0=ALU.mult, op1=ALU.mult)
                nc.vector.scalar_tensor_tensor(
                    t2[:, 0:HD], x_sb[:, hh * D + HD:(hh + 1) * D], s_hh,
                    sw_t[:, 0:HD], op0=ALU.mult, op1=ALU.mult)
                nc.vector.scalar_tensor_tensor(
                    t2[:, HD:D], x_sb[:, hh * D:hh * D + HD], s_hh,
                    sw_t[:, HD:D], op0=ALU.mult, op1=ALU.mult)
                nc.vector.tensor_add(dst[:, hh * D:(hh + 1) * D],
                                     t1[:], t2[:])

        # ================= K projection =================
        wk_sb = w_pool.tile([128, IT * 512], bf16, tag="w", name="wk_sb")
        nc.sync.dma_start(out=wk_sb[:], in_=wkP)
        wv_sb = w_pool.tile([128, IT * 512], bf16, tag="w", name="wv_sb")
        nc.sync.dma_start(out=wv_sb[:], in_=wvP)
        # prefetch first Q weight chunk early (3rd w slot)
        wq_sb0 = w_pool.tile([128, IT * 512], bf16, tag="w", name="wq_sb")
        nc.sync.dma_start(out=wq_sb0[:], in_=wqP[0])

        kTs_sb = kvloc_pool.tile([128, KV * SC], bf16)  # [d, (g s)] local kT
        v_sb = kvloc_pool.tile([128, TT * KV * D], bf16)  # [s,(st hd)] local

        for st in range(TT):
            ps_k = mm_ps.tile([128, KV * D], f32, tag="mm", name="ps_k")
            for it in range(IT):
                nc.tensor.matmul(ps_k[:], hs_tile(it, st),
                                 wk_sb[:, it * 512:(it + 1) * 512],
                                 start=(it == 0), stop=(it == IT - 1))
            krope = rope_pool.tile([128, KV * D], bf16, tag="krope",
                                   name="krope")
            norm_rope(ps_k, KV,
                      cwk_sb[:, st * D:(st + 1) * D],
                      swk_sb[:, st * D:(st + 1) * D], krope)
            for g in range(KV):
                pst = tr_ps.tile([128, 128], bf16, tag="tr", name="pst")
                nc.tensor.transpose(pst[:], krope[:, g * D:(g + 1) * D],
                                    ident[:])
                nc.vector.tensor_copy(
                    kTs_sb[:, g * SC + st * 128: g * SC + (st + 1) * 128],
                    pst[:])

        # ================= V projection =================
        for st in range(TT):
            ps_v = mm_ps.tile([128, KV * D], f32, tag="mm", name="ps_v")
            for it in range(IT):
                nc.tensor.matmul(ps_v[:], hs_tile(it, st),
                                 wv_sb[:, it * 512:(it + 1) * 512],
                                 start=(it == 0), stop=(it == IT - 1))
            nc.vector.tensor_copy(v_sb[:, st * KV * D:(st + 1) * KV * D],
                                  ps_v[:])

        # ---- ship local kT and v, AllGather within group ----
        cc_in = dram_pool.tile([2 * KV * D, SC], bf16)
        cc_out = dram_pool.tile([TPG * 2 * KV * D, SC], bf16)
        nc.sync.dma_start(
            out=cc_in[0:KV * D, :].rearrange("(g p) s -> p g s", p=128),
            in_=kTs_sb.rearrange("p (g s) -> p g s", s=SC))
        nc.sync.dma_start(
            out=cc_in[KV * D:2 * KV * D, :].rearrange("(a p) n -> p a n",
                                                      p=128),
            in_=v_sb.rearrange("p (a n) -> p a n", n=KV * D))
        nc.gpsimd.collective_compute(
            "AllGather", ALU.bypass, replica_groups=GROUPS,
            ins=[cc_in.opt()], outs=[cc_out.opt()])
        ctx_kv.close()

        # ================= Q projection (overlaps collective) ============
        qT_sb = qT_pool.tile([128, H * SC], bf16)   # per head: [d, 512 t]
        for oc in range(OC):
            if oc == 0:
                wq_sb = wq_sb0
            else:
                wq_sb = w_pool.tile([128, IT * 512], bf16, tag="w",
                                    name="wq_sb")
                nc.sync.dma_start(out=wq_sb[:], in_=wqP[oc])
            for tt in range(TT):
                ps_q = mm_ps.tile([128, 512], f32, tag="mm", name="ps_q")
                for it in range(IT):
                    nc.tensor.matmul(ps_q[:], hs_tile(it, tt),
                                     wq_sb[:, it * 512:(it + 1) * 512],
                                     start=(it == 0), stop=(it == IT - 1))
                qrope = rope_pool.tile([128, 4 * D], bf16, tag="qrope",
                                       name="qrope")
                norm_rope(ps_q, 4,
                          cwq_sb[:, tt * D:(tt + 1) * D],
                          swq_sb[:, tt * D:(tt + 1) * D], qrope)
                for hh in range(4):
                    h = oc * 4 + hh
                    pstq = tr_ps.tile([128, 128], bf16, tag="tr", name="pstq")
                    nc.tensor.transpose(pstq[:],
                                        qrope[:, hh * D:(hh + 1) * D],
                                        ident[:])
                    nc.vector.tensor_copy(
                        qT_sb[:, h * SC + tt * 128: h * SC + (tt + 1) * 128],
                        pstq[:])
        ctx_hs.close()

        # ================= load gathered K/V =================
        # cc_out rank rr block of 1024 rows: [0:512] kT (g*128+d, s_local),
        #                                    [512:1024] v (s_local, g*128+d)
        kT_big = kv_pool.tile([128, KV * S], bf16, tag="kT")    # [d,(g rr s)]
        v_big = kv_pool.tile([128, ST * KV * D], bf16, tag="v")  # [s,(st hd)]
        for rr in range(TPG):
            base = rr * 2 * KV * D
            for g in range(KV):
                nc.sync.dma_start(
                    out=kT_big[:,
                               (g * TPG + rr) * SC:(g * TPG + rr + 1) * SC],
                    in_=cc_out[base + g * 128: base + (g + 1) * 128, :])
            nc.sync.dma_start(
                out=v_big[:,
                          rr * TT * KV * D:(rr + 1) * TT * KV * D].rearrange(
                    "p (a n) -> p a n", n=KV * D),
                in_=cc_out[base + KV * D: base + 2 * KV * D, :].rearrange(
                    "(a p) n -> p a n", p=128))

        def kT_tile(g, st):
            # stationary [128 d, 128 s] for s-tile st (st = rr*4 + sub)
            rr, sub = st // TT, st % TT
            off = (g * TPG + rr) * SC + sub * 128
            return kT_big[:, off: off + 128]

        def v_tile(g, st):
            # stationary [128 s, 128 d] for s-tile st
            off = st * KV * D + g * D
            return v_big[:, off: off + D]

        # ================= attention =================
        p2_pool = ep(tc.tile_pool(name="p2", bufs=2))
        attnT_pool = ep(tc.tile_pool(name="attnT", bufs=1))
        den_pool = ep(tc.tile_pool(name="den", bufs=1))
        sc_ps = ep(tc.tile_pool(name="sc_ps", bufs=2, space="PSUM"))
        att_ps = ep(tc.tile_pool(name="att_ps", bufs=4, space="PSUM"))

        attnT_sb = attnT_pool.tile([128, H * SC], bf16)
        for pr in range(H // 2):
            ha = 2 * pr
            g = ha // (H // KV)
            # p2 layout: [128, (st 16, h 2, n 512)] -> exp writes contiguous
            p2 = p2_pool.tile([128, ST * 2 * SC], bf16, tag="p2", name="p2")
            for st in range(ST):
                sc_t = sc_ps.tile([128, 1024], f32, tag="sc", name="sc_t")
                nc.tensor.matmul(sc_t[:, 0:512], kT_tile(g, st),
                                 qT_sb[:, ha * SC:(ha + 1) * SC],
                                 start=True, stop=True)
                nc.tensor.matmul(sc_t[:, 512:1024], kT_tile(g, st),
                                 qT_sb[:, (ha + 1) * SC:(ha + 2) * SC],
                                 start=True, stop=True)
                nc.scalar.activation(p2[:, st * 1024:(st + 1) * 1024],
                                     sc_t[:], AF.Exp)
            ps_att = [att_ps.tile([128, SC], f32, tag="att", name="ps_att")
                      for _ in range(2)]
            for st in range(ST):
                for hh in range(2):
                    nc.tensor.matmul(
                        ps_att[hh][:], v_tile(g, st),
                        p2[:, st * 1024 + hh * 512: st * 1024 + (hh + 1) * 512],
                        start=(st == 0), stop=(st == ST - 1))
            p2v = p2.rearrange("p (a h n) -> p h a n", h=2, n=SC)
            for hh in range(2):
                h = ha + hh
                # denominator: tree-add the 16 exp tiles on DVE, then one
                # ones-matmul for the partition sum (broadcast for free)
                acc = den_pool.tile([128, 4 * SC], bf16, tag="acc",
                                    name="acc")
                nc.vector.tensor_add(acc[:], p2v[:, hh, 0:4, :],
                                     p2v[:, hh, 4:8, :])
                nc.vector.tensor_add(acc[:], acc[:], p2v[:, hh, 8:12, :])
                nc.vector.tensor_add(acc[:], acc[:], p2v[:, hh, 12:16, :])
                t2b = den_pool.tile([128, 2 * SC], bf16, tag="t2b",
                                    name="t2b")
                nc.vector.tensor_add(t2b[:], acc[:, 0:2 * SC],
                                     acc[:, 2 * SC:4 * SC])
                den_b = den_pool.tile([128, SC], bf16, tag="denb",
                                      name="den_b")
                nc.vector.tensor_add(den_b[:], t2b[:, 0:SC], t2b[:, SC:2 * SC])
                ps_db = att_ps.tile([128, SC], f32, tag="att", name="ps_db")
                nc.tensor.matmul(ps_db[:], ones_bf[:], den_b[:],
                                 start=True, stop=True)
                rden = osb_pool.tile([128, SC], f32, tag="osb", name="rden")
                nc.vector.reciprocal_approx_fast(rden[:], ps_db[:])
                nc.vector.tensor_mul(attnT_sb[:, h * SC:(h + 1) * SC],
                                     ps_att[hh][:], rden[:])

        # ================= output projection =================
        for oc in range(OC):
            wo_sb = w_pool.tile([128, IT * 512], bf16, tag="w", name="wo_sb")
            nc.sync.dma_start(out=wo_sb[:], in_=woP[oc])
            for fl in range(4):
                ps_o = att_ps.tile([128, SC], f32, tag="att", name="ps_o")
                for kk in range(IT):
                    nc.tensor.matmul(
                        ps_o[:],
                        wo_sb[:, kk * 512 + fl * 128: kk * 512 + (fl + 1) * 128],
                        attnT_sb[:, kk * SC:(kk + 1) * SC],
                        start=(kk == 0), stop=(kk == IT - 1))
                ft = oc * 4 + fl
                o_sb = osb_pool.tile([128, SC], f32, tag="osb", name="o_sb")
                nc.vector.tensor_copy(o_sb[:], ps_o[:])
                nc.sync.dma_start(out=outT[ft * 128:(ft + 1) * 128, :],
                                  in_=o_sb[:])
    finally:
        ctx.close()
        tc_cm.__exit__(None, None, None)

    nc.compile()
    return nc


def _prep_inputs(hidden_states, cos, sin, Wq, Wk, Wv, Wo, norm_q_w,
                 norm_k_w):
    """Host-side: transpose + bf16-cast weights into tile-major layouts,
    fold norm weights + 1/sqrt(D) into rope coefficients, slice per core."""
    import ml_dtypes
    f = np.float32
    bf = ml_dtypes.bfloat16
    hs = np.asarray(hidden_states, f)
    cos = np.asarray(cos, f)
    sin = np.asarray(sin, f)

    def tile_major(wT, oc_split):
        # wT: [HS, N] -> [oc][128, it, 512] (tile-major over rows)
        n = wT.shape[1]
        arr = wT.reshape(IT, 128, n)
        if oc_split:
            out = np.empty((OC, 128, IT * 512), bf)
            for oc in range(OC):
                blk = arr[:, :, oc * 512:(oc + 1) * 512]  # [it, 128, 512]
                out[oc] = blk.transpose(1, 0, 2).reshape(128, IT * 512)
            return out
        return np.ascontiguousarray(
            arr.transpose(1, 0, 2).reshape(128, IT * 512)).astype(bf)

    wq = tile_major(np.asarray(Wq, f).T, True)       # [4, 128, 8192]
    wk = tile_major(np.asarray(Wk, f).T, False)      # [128, 8192]
    wv = tile_major(np.asarray(Wv, f).T, False)
    wo = tile_major(np.asarray(Wo, f).T, True)
    wqn = np.asarray(norm_q_w, f)
    wkn = np.asarray(norm_k_w, f)

    def rope_consts(w, scale):
        # cw[t, d] = cos[t, d] * w[d] * scale
        # sw[t, d<64]  = -sin[t, d] * w[d+64] * scale
        # sw[t, d>=64] = +sin[t, d] * w[d-64] * scale
        cw = cos * w[None, :] * scale
        w_swap = np.concatenate([w[D // 2:], w[:D // 2]])
        sgn = np.concatenate([-np.ones(D // 2, f), np.ones(D // 2, f)])
        sw = sin * (w_swap * sgn)[None, :] * scale
        return cw.astype(f), sw.astype(f)

    cwq_full, swq_full = rope_consts(wqn, np.float32(D ** -0.5))
    cwk_full, swk_full = rope_consts(wkn, np.float32(1.0))

    def part_major(a):
        # [512, D] -> [128, tt, D] -> [128, tt*D]
        return np.ascontiguousarray(
            a.reshape(TT, 128, D).transpose(1, 0, 2).reshape(128, TT * D))

    in_maps = []
    for c in range(NCORES):
        b, j = divmod(c, TPG)
        sl = slice(j * SC, (j + 1) * SC)
        hsT = hs[b].T[:, sl]                          # [2048 i, 512 t]
        hsp = hsT.reshape(IT, 128, SC).transpose(1, 0, 2).reshape(
            128, IT * SC).astype(bf)
        in_maps.append({
            "hsP": np.ascontiguousarray(hsp),
            "cwq": part_major(cwq_full[sl]),
            "swq": part_major(swq_full[sl]),
            "cwk": part_major(cwk_full[sl]),
            "swk": part_major(swk_full[sl]),
            "wqP": wq, "wkP": wk, "wvP": wv, "woP": wo,
        })
    return in_maps


def _assemble(results):
    out = np.empty((B, S, HS), np.float32)
    for c in range(NCORES):
        b, j = divmod(c, TPG)
        out[b, j * SC:(j + 1) * SC, :] = results[c]["outT"].T
    return out


def kernel(hidden_states, cos, sin, Wq, Wk, Wv, Wo, norm_q_w, norm_k_w,
           _run_kwargs=None):
    from concourse.bass_utils import run_bass_kernel_spmd

    if "nc" not in _BUILT:
        _BUILT["nc"] = _build_program()
    nc = _BUILT["nc"]
    in_maps = _prep_inputs(hidden_states, cos, sin, Wq, Wk, Wv, Wo,
                           norm_q_w, norm_k_w)
    kw = _run_kwargs or {}
    res = run_bass_kernel_spmd(nc, in_maps, list(range(NCORES)), **kw)
    _BUILT["last_results"] = res
    return _assemble(res.results)


# revision 12
# speedup vs baseline: 1.0295x; 1.0295x over previous
"""AceStep GQA attention block on 8 Trainium2 NeuronCores.

Sharding: DP over batch (B=2) x sequence-parallel over S within each batch
group (4 cores each own 512 query positions).  Each core computes K/V for its
own 512 positions, AllGathers K^T/V (bf16) within its 4-core group, then runs
full attention for its query slice and the complete output projection row
block.  No output collective needed - each core owns a distinct slice.

v2 restructure (vs v1 baseline at ~725us):
  - all matmuls bf16 (weights pre-cast + pre-tiled on host, one DMA each)
  - K/V projections first, single AllGather issued early; Q projection and
    its norm/rope overlap the collective in flight
  - rmsnorm+rope fused: tensor_tensor_reduce (square+sum in one DVE op) +
    scalar_tensor_tensor ((psum*scale)*rope_coeff in one DVE op)
  - softmax denominator off the tensor engine: DVE tree-adds of exp tiles +
    one ones-matmul per head for the partition-sum broadcast; 1/den via
    DVE reciprocal_approx_fast.  Scalar engine runs only Exp during
    attention (no activation-table thrash).
  - exp batched per head-pair ([128,1024] per s-tile), p-tile buffer
    double-buffered so ACT never waits on the AV matmuls.

Device layouts (per core, c = 4*b + j):
  hsP   [128, it 16, t 512]  hidden[b].T tile-major (partition = i % 128)
  wkP/wvP [128, it 16, n 512]   W^T tile-major
  wqP/woP [oc 4][128, it/kk 16, n 512]
  cw*/sw* [128, tt 4, d 128]  rope coeffs (norm weight + scale folded in)
  outT  [HS, SC] f32
"""

import numpy as np

H, KV, D = 16, 4, 128
HD = D // 2
B, S, HS = 2, 2048, 2048
EPS = 1e-6
NCORES = 8
TPG = 4              # cores per batch group (sequence split)
SC = S // TPG        # 512 sequence positions per core
TT = SC // 128       # 4 t-tiles per core
IT = HS // 128       # 16 contraction tiles
ST = S // 128        # 16 s-tiles (full sequence)
OC = 4               # 512-wide output chunks for Q/O projections
GROUPS = [[0, 1, 2, 3], [4, 5, 6, 7]]

_BUILT = {}


def _build_program():
    from contextlib import ExitStack

    import concourse.bass as bass
    import concourse.bacc as bacc
    import concourse.mybir as mybir
    import concourse.tile as tile
    from concourse.masks import make_identity

    f32 = mybir.dt.float32
    bf16 = mybir.dt.bfloat16
    AF = mybir.ActivationFunctionType
    ALU = mybir.AluOpType

    nc = bacc.Bacc("TRN2", target_bir_lowering=False, debug=False,
                   num_devices=NCORES)

    # ---- external I/O (per core) ----
    hsP = nc.dram_tensor("hsP", [128, IT * SC], bf16, kind="ExternalInput").ap()
    wkP = nc.dram_tensor("wkP", [128, IT * 512], bf16, kind="ExternalInput").ap()
    wvP = nc.dram_tensor("wvP", [128, IT * 512], bf16, kind="ExternalInput").ap()
    wqP = nc.dram_tensor("wqP", [OC, 128, IT * 512], bf16,
                         kind="ExternalInput").ap()
    woP = nc.dram_tensor("woP", [OC, 128, IT * 512], bf16,
                         kind="ExternalInput").ap()
    cwq = nc.dram_tensor("cwq", [128, TT * D], f32, kind="ExternalInput").ap()
    swq = nc.dram_tensor("swq", [128, TT * D], f32, kind="ExternalInput").ap()
    cwk = nc.dram_tensor("cwk", [128, TT * D], f32, kind="ExternalInput").ap()
    swk = nc.dram_tensor("swk", [128, TT * D], f32, kind="ExternalInput").ap()
    outT = nc.dram_tensor("outT", [HS, SC], f32, kind="ExternalOutput").ap()

    tc_cm = tile.TileContext(nc)
    ctx = ExitStack()
    tc = tc_cm.__enter__()
    try:
        ep = ctx.enter_context
        const_pool = ep(tc.tile_pool(name="const", bufs=1))
        w_pool = ep(tc.tile_pool(name="w", bufs=2))
        scr_pool = ep(tc.tile_pool(name="scr", bufs=2))
        rope_pool = ep(tc.tile_pool(name="rope", bufs=2))
        qT_pool = ep(tc.tile_pool(name="qT", bufs=1))
        kv_pool = ep(tc.tile_pool(name="kv", bufs=1))
        osb_pool = ep(tc.tile_pool(name="osb", bufs=3))
        dram_pool = ep(tc.tile_pool(name="dram", bufs=1, space="DRAM"))

        # scoped pools released before the attention phase needs the space
        # (pool releases must be LIFO: kvloc closes first, then hs)
        ctx_hs = ExitStack()      # hs + projection psum (until Q proj done)
        ctx_kv = ExitStack()      # local kT/v staging (until pack DMAs done)
        hs_pool = ctx_hs.enter_context(tc.tile_pool(name="hs", bufs=1))
        mm_ps = ctx_hs.enter_context(
            tc.tile_pool(name="mm_ps", bufs=5, space="PSUM"))
        tr_ps = ctx_hs.enter_context(
            tc.tile_pool(name="tr_ps", bufs=2, space="PSUM"))
        kvloc_pool = ctx_kv.enter_context(tc.tile_pool(name="kvloc", bufs=1))

        # ---- constants ----
        ident = const_pool.tile([128, 128], bf16)
        make_identity(nc, ident)
        ones_bf = const_pool.tile([128, 128], bf16)
        nc.vector.memset(ones_bf, 1.0)
        eps_sb = const_pool.tile([128, 1], f32)
        nc.vector.memset(eps_sb, EPS)

        cwq_sb = const_pool.tile([128, TT * D], f32)
        swq_sb = const_pool.tile([128, TT * D], f32)
        cwk_sb = const_pool.tile([128, TT * D], f32)
        swk_sb = const_pool.tile([128, TT * D], f32)
        for (dst, src) in ((cwq_sb, cwq), (swq_sb, swq),
                           (cwk_sb, cwk), (swk_sb, swk)):
            nc.sync.dma_start(out=dst[:], in_=src)

        hs_sb = hs_pool.tile([128, IT * SC], bf16)
        nc.sync.dma_start(out=hs_sb[:], in_=hsP)

        def hs_tile(it, tt):
            # stationary [128 i, 128 t]
            off = it * SC + tt * 128
            return hs_sb[:, off: off + 128]

        def squares(ps, nh, ssum, base):
            """sum(x^2) over D per head via ACT Square + accum_out."""
            for hh in range(nh):
                sqd = scr_pool.tile([128, D], f32, tag="sqd", name="sqd")
                nc.scalar.activation(
                    sqd[:], ps[:, hh * D:(hh + 1) * D], AF.Square,
                    accum_out=ssum[:, base + hh:base + hh + 1])

        def scales_of(ssum, n):
            """rsqrt(ssum/D + eps) batched: one Ln + one Exp table load."""
            lnm = scr_pool.tile([128, n], f32, tag="lnm", name="lnm")
            nc.scalar.activation(lnm[:], ssum[:], AF.Ln, bias=eps_sb[:],
                                 scale=1.0 / D)
            sc_t = scr_pool.tile([128, n], f32, tag="sct", name="sc_t")
            nc.scalar.activation(sc_t[:], lnm[:], AF.Exp, scale=-0.5)
            return sc_t

        def rope_apply(ps, nh, sc_sl, cw_t, sw_t, dst):
            """ps: psum [128 t, nh*D]; sc_sl: [128, nh] scales;
            dst: bf16 [128, nh*D].  All heads batched per DVE op."""
            scf = scr_pool.tile([128, nh * D], bf16, tag="scf", name="scf")
            nc.vector.tensor_copy(
                scf.rearrange("p (h d) -> p h d", d=D),
                sc_sl.rearrange("p (h one) -> p h one", one=1).broadcast_to(
                    [128, nh, D]))
            xs = scr_pool.tile([128, nh * D], bf16, tag="xs", name="xs")
            nc.vector.tensor_mul(xs[:], ps[:], scf[:])
            t1 = scr_pool.tile([128, nh * D], bf16, tag="t1", name="t1")
            cwb = cw_t.rearrange("p (one d) -> p one d", one=1).broadcast_to(
                [128, nh, D])
            nc.vector.tensor_mul(t1.rearrange("p (h d) -> p h d", d=D),
                                 xs.rearrange("p (h d) -> p h d", d=D), cwb)
            t2 = scr_pool.tile([128, nh * D], bf16, tag="t2", name="t2")
            xsv = xs.rearrange("p (h two x) -> p h two x", two=2, x=HD)
            t2v = t2.rearrange("p (h two x) -> p h two x", two=2, x=HD)
            swb_lo = sw_t[:, 0:HD].rearrange(
                "p (one x) -> p one x", one=1).broadcast_to([128, nh, HD])
            swb_hi = sw_t[:, HD:D].rearrange(
                "p (one x) -> p one x", one=1).broadcast_to([128, nh, HD])
            nc.vector.tensor_mul(t2v[:, :, 0, :], xsv[:, :, 1, :], swb_lo)
            nc.vector.tensor_mul(t2v[:, :, 1, :], xsv[:, :, 0, :], swb_hi)
            nc.vector.tensor_add(dst[:], t1[:], t2[:])

        # ================= K projection =================
        wk_sb = w_pool.tile([128, IT * 512], bf16, tag="w", name="wk_sb")
        nc.sync.dma_start(out=wk_sb[:], in_=wkP)
        wv_sb = w_pool.tile([128, IT * 512], bf16, tag="w", name="wv_sb")
        nc.sync.dma_start(out=wv_sb[:], in_=wvP)
        # prefetch first Q weight chunk early (3rd w slot)
        wq_sb0 = w_pool.tile([128, IT * 512], bf16, tag="w", name="wq_sb")
        nc.sync.dma_start(out=wq_sb0[:], in_=wqP[0])

        kTs_sb = kvloc_pool.tile([128, KV * SC], bf16)  # [d, (g s)] local kT
        v_sb = kvloc_pool.tile([128, TT * KV * D], bf16)  # [s,(st hd)] local

        ps_ks = []
        ssum_k = scr_pool.tile([128, TT * KV], f32, tag="ssum", name="ssum_k")
        for st in range(TT):
            ps_k = mm_ps.tile([128, KV * D], f32, tag="mm", name="ps_k")
            for it in range(IT):
                nc.tensor.matmul(ps_k[:], hs_tile(it, st),
                                 wk_sb[:, it * 512:(it + 1) * 512],
                                 start=(it == 0), stop=(it == IT - 1))
            squares(ps_k, KV, ssum_k, st * KV)
            ps_ks.append(ps_k)
        sck = scales_of(ssum_k, TT * KV)
        kTs_v = kTs_sb.rearrange("p (g s) -> p g s", s=SC)
        for st in range(TT):
            krope = rope_pool.tile([128, KV * D], bf16, tag="krope",
                                   name="krope")
            rope_apply(ps_ks[st], KV, sck[:, st * KV:(st + 1) * KV],
                       cwk_sb[:, st * D:(st + 1) * D],
                       swk_sb[:, st * D:(st + 1) * D], krope)
            pst = tr_ps.tile([128, KV * 128], bf16, tag="tr", name="pst")
            for g in range(KV):
                nc.tensor.transpose(pst[:, g * 128:(g + 1) * 128],
                                    krope[:, g * D:(g + 1) * D], ident[:])
            nc.vector.tensor_copy(
                kTs_v[:, :, st * 128:(st + 1) * 128],
                pst.rearrange("p (g t) -> p g t", t=128))

        # ================= V projection =================
        for st in range(TT):
            ps_v = mm_ps.tile([128, KV * D], f32, tag="mm", name="ps_v")
            for it in range(IT):
                nc.tensor.matmul(ps_v[:], hs_tile(it, st),
                                 wv_sb[:, it * 512:(it + 1) * 512],
                                 start=(it == 0), stop=(it == IT - 1))
            nc.vector.tensor_copy(v_sb[:, st * KV * D:(st + 1) * KV * D],
                                  ps_v[:])

        # ---- ship local kT and v, AllGather within group ----
        cc_in = dram_pool.tile([2 * KV * D, SC], bf16)
        cc_out = dram_pool.tile([TPG * 2 * KV * D, SC], bf16)
        nc.sync.dma_start(
            out=cc_in[0:KV * D, :].rearrange("(g p) s -> p g s", p=128),
            in_=kTs_sb.rearrange("p (g s) -> p g s", s=SC))
        nc.sync.dma_start(
            out=cc_in[KV * D:2 * KV * D, :].rearrange("(a p) n -> p a n",
                                                      p=128),
            in_=v_sb.rearrange("p (a n) -> p a n", n=KV * D))
        nc.gpsimd.collective_compute(
            "AllGather", ALU.bypass, replica_groups=GROUPS,
            ins=[cc_in.opt()], outs=[cc_out.opt()])
        ctx_kv.close()

        # ================= Q projection (overlaps collective) ============
        qT_sb = qT_pool.tile([128, H * SC], bf16)   # per head: [d, 512 t]
        for oc in range(OC):
            if oc == 0:
                wq_sb = wq_sb0
            else:
                wq_sb = w_pool.tile([128, IT * 512], bf16, tag="w",
                                    name="wq_sb")
                nc.sync.dma_start(out=wq_sb[:], in_=wqP[oc])
            ps_qs = []
            ssum_q = scr_pool.tile([128, TT * 4], f32, tag="ssum",
                                   name="ssum_q")
            for tt in range(TT):
                ps_q = mm_ps.tile([128, 512], f32, tag="mm", name="ps_q")
                for it in range(IT):
                    nc.tensor.matmul(ps_q[:], hs_tile(it, tt),
                                     wq_sb[:, it * 512:(it + 1) * 512],
                                     start=(it == 0), stop=(it == IT - 1))
                squares(ps_q, 4, ssum_q, tt * 4)
                ps_qs.append(ps_q)
            scq = scales_of(ssum_q, TT * 4)
            qT_v = qT_sb.rearrange("p (h t) -> p h t", t=SC)
            for tt in range(TT):
                qrope = rope_pool.tile([128, 4 * D], bf16, tag="qrope",
                                       name="qrope")
                rope_apply(ps_qs[tt], 4, scq[:, tt * 4:(tt + 1) * 4],
                           cwq_sb[:, tt * D:(tt + 1) * D],
                           swq_sb[:, tt * D:(tt + 1) * D], qrope)
                pstq = tr_ps.tile([128, 4 * 128], bf16, tag="tr", name="pstq")
                for hh in range(4):
                    nc.tensor.transpose(pstq[:, hh * 128:(hh + 1) * 128],
                                        qrope[:, hh * D:(hh + 1) * D],
                                        ident[:])
                nc.vector.tensor_copy(
                    qT_v[:, oc * 4:(oc + 1) * 4, tt * 128:(tt + 1) * 128],
                    pstq.rearrange("p (h t) -> p h t", t=128))
        ctx_hs.close()

        # ================= load gathered K/V =================
        # cc_out rank rr block of 1024 rows: [0:512] kT (g*128+d, s_local),
        #                                    [512:1024] v (s_local, g*128+d)
        kT_big = kv_pool.tile([128, KV * S], bf16, tag="kT")    # [d,(g rr s)]
        v_big = kv_pool.tile([128, ST * KV * D], bf16, tag="v")  # [s,(st hd)]
        kT_bv = kT_big.rearrange("p (g r s) -> p g r s", r=TPG, s=SC)
        for rr in range(TPG):
            base = rr * 2 * KV * D
            nc.sync.dma_start(
                out=kT_bv[:, :, rr, :],
                in_=cc_out[base: base + KV * D, :].rearrange(
                    "(g p) s -> p g s", p=128))
            nc.sync.dma_start(
                out=v_big[:,
                          rr * TT * KV * D:(rr + 1) * TT * KV * D].rearrange(
                    "p (a n) -> p a n", n=KV * D),
                in_=cc_out[base + KV * D: base + 2 * KV * D, :].rearrange(
                    "(a p) n -> p a n", p=128))

        def kT_tile(g, st):
            # stationary [128 d, 128 s] for s-tile st (st = rr*4 + sub)
            rr, sub = st // TT, st % TT
            off = (g * TPG + rr) * SC + sub * 128
            return kT_big[:, off: off + 128]

        def v_tile(g, st):
            # stationary [128 s, 128 d] for s-tile st
            off = st * KV * D + g * D
            return v_big[:, off: off + D]

        # ================= attention =================
        p2_pool = ep(tc.tile_pool(name="p2", bufs=2))
        attnT_pool = ep(tc.tile_pool(name="attnT", bufs=1))
        den_pool = ep(tc.tile_pool(name="den", bufs=1))
        sc_ps = ep(tc.tile_pool(name="sc_ps", bufs=2, space="PSUM"))
        att_ps = ep(tc.tile_pool(name="att_ps", bufs=4, space="PSUM"))

        attnT_sb = attnT_pool.tile([128, H * SC], bf16)
        for pr in range(H // 2):
            ha = 2 * pr
            g = ha // (H // KV)
            # p2 layout: [128, (st 16, h 2, n 512)] -> exp writes contiguous
            p2 = p2_pool.tile([128, ST * 2 * SC], bf16, tag="p2", name="p2")
            for st in range(ST):
                sc_t = sc_ps.tile([128, 1024], f32, tag="sc", name="sc_t")
                nc.tensor.matmul(sc_t[:, 0:512], kT_tile(g, st),
                                 qT_sb[:, ha * SC:(ha + 1) * SC],
                                 start=True, stop=True)
                nc.tensor.matmul(sc_t[:, 512:1024], kT_tile(g, st),
                                 qT_sb[:, (ha + 1) * SC:(ha + 2) * SC],
                                 start=True, stop=True)
                nc.scalar.activation(p2[:, st * 1024:(st + 1) * 1024],
                                     sc_t[:], AF.Exp)
            ps_att = [att_ps.tile([128, SC], f32, tag="att", name="ps_att")
                      for _ in range(2)]
            for st in range(ST):
                for hh in range(2):
                    nc.tensor.matmul(
                        ps_att[hh][:], v_tile(g, st),
                        p2[:, st * 1024 + hh * 512: st * 1024 + (hh + 1) * 512],
                        start=(st == 0), stop=(st == ST - 1))
            p2v = p2.rearrange("p (a h n) -> p h a n", h=2, n=SC)
            for hh in range(2):
                h = ha + hh
                # denominator: tree-add the 16 exp tiles on DVE, then one
                # ones-matmul for the partition sum (broadcast for free)
                acc = den_pool.tile([128, 4 * SC], bf16, tag="acc",
                                    name="acc")
                nc.vector.tensor_add(acc[:], p2v[:, hh, 0:4, :],
                                     p2v[:, hh, 4:8, :])
                nc.vector.tensor_add(acc[:], acc[:], p2v[:, hh, 8:12, :])
                nc.vector.tensor_add(acc[:], acc[:], p2v[:, hh, 12:16, :])
                t2b = den_pool.tile([128, 2 * SC], bf16, tag="t2b",
                                    name="t2b")
                nc.vector.tensor_add(t2b[:], acc[:, 0:2 * SC],
                                     acc[:, 2 * SC:4 * SC])
                den_b = den_pool.tile([128, SC], bf16, tag="denb",
                                      name="den_b")
                nc.vector.tensor_add(den_b[:], t2b[:, 0:SC], t2b[:, SC:2 * SC])
                ps_db = att_ps.tile([128, SC], f32, tag="att", name="ps_db")
                nc.tensor.matmul(ps_db[:], ones_bf[:], den_b[:],
                                 start=True, stop=True)
                rden = osb_pool.tile([128, SC], f32, tag="osb", name="rden")
                nc.vector.reciprocal_approx_fast(rden[:], ps_db[:])
                nc.vector.tensor_mul(attnT_sb[:, h * SC:(h + 1) * SC],
                                     ps_att[hh][:], rden[:])

        # ================= output projection =================
        for oc in range(OC):
            wo_sb = w_pool.tile([128, IT * 512], bf16, tag="w", name="wo_sb")
            nc.sync.dma_start(out=wo_sb[:], in_=woP[oc])
            for fl in range(4):
                ps_o = att_ps.tile([128, SC], f32, tag="att", name="ps_o")
                for kk in range(IT):
                    nc.tensor.matmul(
                        ps_o[:],
                        wo_sb[:, kk * 512 + fl * 128: kk * 512 + (fl + 1) * 128],
                        attnT_sb[:, kk * SC:(kk + 1) * SC],
                        start=(kk == 0), stop=(kk == IT - 1))
                ft = oc * 4 + fl
                o_sb = osb_pool.tile([128, SC], f32, tag="osb", name="o_sb")
                nc.vector.tensor_copy(o_sb[:], ps_o[:])
                nc.sync.dma_start(out=outT[ft * 128:(ft + 1) * 128, :],
                                  in_=o_sb[:])
    finally:
        ctx.close()
        tc_cm.__exit__(None, None, None)

    nc.compile()
    return nc


def _prep_inputs(hidden_states, cos, sin, Wq, Wk, Wv, Wo, norm_q_w,
                 norm_k_w):
    """Host-side: transpose + bf16-cast weights into tile-major layouts,
    fold norm weights + 1/sqrt(D) into rope coefficients, slice per core."""
    import ml_dtypes
    f = np.float32
    bf = ml_dtypes.bfloat16
    hs = np.asarray(hidden_states, f)
    cos = np.asarray(cos, f)
    sin = np.asarray(sin, f)

    def tile_major(wT, oc_split):
        # wT: [HS, N] -> [oc][128, it, 512] (tile-major over rows)
        n = wT.shape[1]
        arr = wT.reshape(IT, 128, n)
        if oc_split:
            out = np.empty((OC, 128, IT * 512), bf)
            for oc in range(OC):
                blk = arr[:, :, oc * 512:(oc + 1) * 512]  # [it, 128, 512]
                out[oc] = blk.transpose(1, 0, 2).reshape(128, IT * 512)
            return out
        return np.ascontiguousarray(
            arr.transpose(1, 0, 2).reshape(128, IT * 512)).astype(bf)

    wq = tile_major(np.asarray(Wq, f).T, True)       # [4, 128, 8192]
    wk = tile_major(np.asarray(Wk, f).T, False)      # [128, 8192]
    wv = tile_major(np.asarray(Wv, f).T, False)
    wo = tile_major(np.asarray(Wo, f).T, True)
    wqn = np.asarray(norm_q_w, f)
    wkn = np.asarray(norm_k_w, f)

    def rope_consts(w, scale):
        # cw[t, d] = cos[t, d] * w[d] * scale
        # sw[t, d<64]  = -sin[t, d] * w[d+64] * scale
        # sw[t, d>=64] = +sin[t, d] * w[d-64] * scale
        cw = cos * w[None, :] * scale
        w_swap = np.concatenate([w[D // 2:], w[:D // 2]])
        sgn = np.concatenate([-np.ones(D // 2, f), np.ones(D // 2, f)])
        sw = sin * (w_swap * sgn)[None, :] * scale
        return cw.astype(f), sw.astype(f)

    cwq_full, swq_full = rope_consts(wqn, np.float32(D ** -0.5))
    cwk_full, swk_full = rope_consts(wkn, np.float32(1.0))

    def part_major(a):
        # [512, D] -> [128, tt, D] -> [128, tt*D]
        return np.ascontiguousarray(
            a.reshape(TT, 128, D).transpose(1, 0, 2).reshape(128, TT * D))

    in_maps = []
    for c in range(NCORES):
        b, j = divmod(c, TPG)
        sl = slice(j * SC, (j + 1) * SC)
        hsT = hs[b].T[:, sl]                          # [2048 i, 512 t]
        hsp = hsT.reshape(IT, 128, SC).transpose(1, 0, 2).reshape(
            128, IT * SC).astype(bf)
        in_maps.append({
            "hsP": np.ascontiguousarray(hsp),
            "cwq": part_major(cwq_full[sl]),
            "swq": part_major(swq_full[sl]),
            "cwk": part_major(cwk_full[sl]),
            "swk": part_major(swk_full[sl]),
            "wqP": wq, "wkP": wk, "wvP": wv, "woP": wo,
        })
    return in_maps


def _assemble(results):
    out = np.empty((B, S, HS), np.float32)
    for c in range(NCORES):
        b, j = divmod(c, TPG)
        out[b, j * SC:(j + 1) * SC, :] = results[c]["outT"].T
    return out


def kernel(hidden_states, cos, sin, Wq, Wk, Wv, Wo, norm_q_w, norm_k_w,
           _run_kwargs=None):
    from concourse.bass_utils import run_bass_kernel_spmd

    if "nc" not in _BUILT:
        _BUILT["nc"] = _build_program()
    nc = _BUILT["nc"]
    in_maps = _prep_inputs(hidden_states, cos, sin, Wq, Wk, Wv, Wo,
                           norm_q_w, norm_k_w)
    kw = _run_kwargs or {}
    res = run_bass_kernel_spmd(nc, in_maps, list(range(NCORES)), **kw)
    _BUILT["last_results"] = res
    return _assemble(res.results)


# revision 14
# speedup vs baseline: 1.0613x; 1.0309x over previous
"""AceStep GQA attention block on 8 Trainium2 NeuronCores.

Sharding: DP over batch (B=2) x sequence-parallel over S within each batch
group (4 cores each own 512 query positions).  Each core computes K/V for its
own 512 positions, AllGathers K^T/V (bf16) within its 4-core group, then runs
full attention for its query slice and the complete output projection row
block.  No output collective needed - each core owns a distinct slice.

v2 restructure (vs v1 baseline at ~725us):
  - all matmuls bf16 (weights pre-cast + pre-tiled on host, one DMA each)
  - K/V projections first, single AllGather issued early; Q projection and
    its norm/rope overlap the collective in flight
  - rmsnorm+rope fused: tensor_tensor_reduce (square+sum in one DVE op) +
    scalar_tensor_tensor ((psum*scale)*rope_coeff in one DVE op)
  - softmax denominator off the tensor engine: DVE tree-adds of exp tiles +
    one ones-matmul per head for the partition-sum broadcast; 1/den via
    DVE reciprocal_approx_fast.  Scalar engine runs only Exp during
    attention (no activation-table thrash).
  - exp batched per head-pair ([128,1024] per s-tile), p-tile buffer
    double-buffered so ACT never waits on the AV matmuls.

Device layouts (per core, c = 4*b + j):
  hsP   [128, it 16, t 512]  hidden[b].T tile-major (partition = i % 128)
  wkP/wvP [128, it 16, n 512]   W^T tile-major
  wqP/woP [oc 4][128, it/kk 16, n 512]
  cw*/sw* [128, tt 4, d 128]  rope coeffs (norm weight + scale folded in)
  outT  [HS, SC] f32
"""

import numpy as np

H, KV, D = 16, 4, 128
HD = D // 2
B, S, HS = 2, 2048, 2048
EPS = 1e-6
NCORES = 8
TPG = 4              # cores per batch group (sequence split)
SC = S // TPG        # 512 sequence positions per core
TT = SC // 128       # 4 t-tiles per core
IT = HS // 128       # 16 contraction tiles
ST = S // 128        # 16 s-tiles (full sequence)
OC = 4               # 512-wide output chunks for Q/O projections
GROUPS = [[0, 1, 2, 3], [4, 5, 6, 7]]

_BUILT = {}


def _build_program():
    from contextlib import ExitStack

    import concourse.bass as bass
    import concourse.bacc as bacc
    import concourse.mybir as mybir
    import concourse.tile as tile
    from concourse.masks import make_identity

    f32 = mybir.dt.float32
    bf16 = mybir.dt.bfloat16
    AF = mybir.ActivationFunctionType
    ALU = mybir.AluOpType

    nc = bacc.Bacc("TRN2", target_bir_lowering=False, debug=False,
                   num_devices=NCORES)

    # ---- external I/O (per core) ----
    hsP = nc.dram_tensor("hsP", [128, IT * SC], bf16, kind="ExternalInput").ap()
    wkP = nc.dram_tensor("wkP", [128, IT * 512], bf16, kind="ExternalInput").ap()
    wvP = nc.dram_tensor("wvP", [128, IT * 512], bf16, kind="ExternalInput").ap()
    wqP = nc.dram_tensor("wqP", [OC, 128, IT * 512], bf16,
                         kind="ExternalInput").ap()
    woP = nc.dram_tensor("woP", [OC, 128, IT * 512], bf16,
                         kind="ExternalInput").ap()
    cwq = nc.dram_tensor("cwq", [128, TT * D], f32, kind="ExternalInput").ap()
    swq = nc.dram_tensor("swq", [128, TT * D], f32, kind="ExternalInput").ap()
    cwk = nc.dram_tensor("cwk", [128, TT * D], f32, kind="ExternalInput").ap()
    swk = nc.dram_tensor("swk", [128, TT * D], f32, kind="ExternalInput").ap()
    outT = nc.dram_tensor("outT", [HS, SC], f32, kind="ExternalOutput").ap()

    tc_cm = tile.TileContext(nc)
    ctx = ExitStack()
    tc = tc_cm.__enter__()
    try:
        ep = ctx.enter_context
        const_pool = ep(tc.tile_pool(name="const", bufs=1))
        w_pool = ep(tc.tile_pool(name="w", bufs=2))
        scr_pool = ep(tc.tile_pool(name="scr", bufs=2))
        rope_pool = ep(tc.tile_pool(name="rope", bufs=2))
        qT_pool = ep(tc.tile_pool(name="qT", bufs=1))
        kv_pool = ep(tc.tile_pool(name="kv", bufs=1))
        osb_pool = ep(tc.tile_pool(name="osb", bufs=3))
        dram_pool = ep(tc.tile_pool(name="dram", bufs=1, space="DRAM"))

        # scoped pools released before the attention phase needs the space
        # (pool releases must be LIFO: kvloc closes first, then hs)
        ctx_hs = ExitStack()      # hs + projection psum (until Q proj done)
        ctx_kv = ExitStack()      # local kT/v staging (until pack DMAs done)
        hs_pool = ctx_hs.enter_context(tc.tile_pool(name="hs", bufs=1))
        mm_ps = ctx_hs.enter_context(
            tc.tile_pool(name="mm_ps", bufs=5, space="PSUM"))
        tr_ps = ctx_hs.enter_context(
            tc.tile_pool(name="tr_ps", bufs=2, space="PSUM"))
        kvloc_pool = ctx_kv.enter_context(tc.tile_pool(name="kvloc", bufs=1))

        # ---- constants ----
        ident = const_pool.tile([128, 128], bf16)
        make_identity(nc, ident)
        ones_bf = const_pool.tile([128, 128], bf16)
        nc.vector.memset(ones_bf, 1.0)
        eps_sb = const_pool.tile([128, 1], f32)
        nc.vector.memset(eps_sb, EPS)

        cwq_sb = const_pool.tile([128, TT * D], f32)
        swq_sb = const_pool.tile([128, TT * D], f32)
        cwk_sb = const_pool.tile([128, TT * D], f32)
        swk_sb = const_pool.tile([128, TT * D], f32)
        for (dst, src) in ((cwq_sb, cwq), (swq_sb, swq),
                           (cwk_sb, cwk), (swk_sb, swk)):
            nc.sync.dma_start(out=dst[:], in_=src)

        hs_sb = hs_pool.tile([128, IT * SC], bf16)
        nc.sync.dma_start(out=hs_sb[:], in_=hsP)

        def hs_tile(it, tt):
            # stationary [128 i, 128 t]
            off = it * SC + tt * 128
            return hs_sb[:, off: off + 128]

        def squares(ps, nh, ssum, base):
            """sum(x^2) over D per head via ACT Square + accum_out."""
            for hh in range(nh):
                sqd = scr_pool.tile([128, D], f32, tag="sqd", name="sqd")
                nc.scalar.activation(
                    sqd[:], ps[:, hh * D:(hh + 1) * D], AF.Square,
                    accum_out=ssum[:, base + hh:base + hh + 1])

        def scales_of(ssum, n):
            """rsqrt(ssum/D + eps) batched: one Ln + one Exp table load."""
            lnm = scr_pool.tile([128, n], f32, tag="lnm", name="lnm")
            nc.scalar.activation(lnm[:], ssum[:], AF.Ln, bias=eps_sb[:],
                                 scale=1.0 / D)
            sc_t = scr_pool.tile([128, n], f32, tag="sct", name="sc_t")
            nc.scalar.activation(sc_t[:], lnm[:], AF.Exp, scale=-0.5)
            return sc_t

        def rope_apply(ps, nh, sc_sl, cw_t, sw_t, dst):
            """ps: psum [128 t, nh*D]; sc_sl: [128, nh] scales;
            dst: bf16 [128, nh*D].  All heads batched per DVE op."""
            scf = scr_pool.tile([128, nh * D], bf16, tag="scf", name="scf")
            nc.vector.tensor_copy(
                scf.rearrange("p (h d) -> p h d", d=D),
                sc_sl.rearrange("p (h one) -> p h one", one=1).broadcast_to(
                    [128, nh, D]))
            xs = scr_pool.tile([128, nh * D], bf16, tag="xs", name="xs")
            nc.vector.tensor_mul(xs[:], ps[:], scf[:])
            t1 = scr_pool.tile([128, nh * D], bf16, tag="t1", name="t1")
            cwb = cw_t.rearrange("p (one d) -> p one d", one=1).broadcast_to(
                [128, nh, D])
            nc.vector.tensor_mul(t1.rearrange("p (h d) -> p h d", d=D),
                                 xs.rearrange("p (h d) -> p h d", d=D), cwb)
            t2 = scr_pool.tile([128, nh * D], bf16, tag="t2", name="t2")
            xsv = xs.rearrange("p (h two x) -> p h two x", two=2, x=HD)
            t2v = t2.rearrange("p (h two x) -> p h two x", two=2, x=HD)
            swb_lo = sw_t[:, 0:HD].rearrange(
                "p (one x) -> p one x", one=1).broadcast_to([128, nh, HD])
            swb_hi = sw_t[:, HD:D].rearrange(
                "p (one x) -> p one x", one=1).broadcast_to([128, nh, HD])
            nc.vector.tensor_mul(t2v[:, :, 0, :], xsv[:, :, 1, :], swb_lo)
            nc.vector.tensor_mul(t2v[:, :, 1, :], xsv[:, :, 0, :], swb_hi)
            nc.vector.tensor_add(dst[:], t1[:], t2[:])

        # ================= K projection =================
        wk_sb = w_pool.tile([128, IT * 512], bf16, tag="w", name="wk_sb")
        nc.sync.dma_start(out=wk_sb[:], in_=wkP)
        wv_sb = w_pool.tile([128, IT * 512], bf16, tag="w", name="wv_sb")
        nc.sync.dma_start(out=wv_sb[:], in_=wvP)
        # prefetch first Q weight chunk early (3rd w slot)
        wq_sb0 = w_pool.tile([128, IT * 512], bf16, tag="w", name="wq_sb")
        nc.sync.dma_start(out=wq_sb0[:], in_=wqP[0])

        kTs_sb = kvloc_pool.tile([128, KV * SC], bf16)  # [d, (g s)] local kT
        v_sb = kvloc_pool.tile([128, TT * KV * D], bf16)  # [s,(st hd)] local

        ps_ks = []
        ssum_k = scr_pool.tile([128, TT * KV], f32, tag="ssum", name="ssum_k")
        for st in range(TT):
            ps_k = mm_ps.tile([128, KV * D], f32, tag="mm", name="ps_k")
            for it in range(IT):
                nc.tensor.matmul(ps_k[:], hs_tile(it, st),
                                 wk_sb[:, it * 512:(it + 1) * 512],
                                 start=(it == 0), stop=(it == IT - 1))
            squares(ps_k, KV, ssum_k, st * KV)
            ps_ks.append(ps_k)
        sck = scales_of(ssum_k, TT * KV)
        kTs_v = kTs_sb.rearrange("p (g s) -> p g s", s=SC)
        for st in range(TT):
            krope = rope_pool.tile([128, KV * D], bf16, tag="krope",
                                   name="krope")
            rope_apply(ps_ks[st], KV, sck[:, st * KV:(st + 1) * KV],
                       cwk_sb[:, st * D:(st + 1) * D],
                       swk_sb[:, st * D:(st + 1) * D], krope)
            pst = tr_ps.tile([128, KV * 128], bf16, tag="tr", name="pst")
            for g in range(KV):
                nc.tensor.transpose(pst[:, g * 128:(g + 1) * 128],
                                    krope[:, g * D:(g + 1) * D], ident[:])
            nc.vector.tensor_copy(
                kTs_v[:, :, st * 128:(st + 1) * 128],
                pst.rearrange("p (g t) -> p g t", t=128))

        # ================= V projection =================
        for st in range(TT):
            ps_v = mm_ps.tile([128, KV * D], f32, tag="mm", name="ps_v")
            for it in range(IT):
                nc.tensor.matmul(ps_v[:], hs_tile(it, st),
                                 wv_sb[:, it * 512:(it + 1) * 512],
                                 start=(it == 0), stop=(it == IT - 1))
            nc.vector.tensor_copy(v_sb[:, st * KV * D:(st + 1) * KV * D],
                                  ps_v[:])

        # ---- ship local kT and v, AllGather within group ----
        cc_in = dram_pool.tile([2 * KV * D, SC], bf16)
        cc_out = dram_pool.tile([TPG * 2 * KV * D, SC], bf16)
        nc.sync.dma_start(
            out=cc_in[0:KV * D, :].rearrange("(g p) s -> p g s", p=128),
            in_=kTs_sb.rearrange("p (g s) -> p g s", s=SC))
        nc.sync.dma_start(
            out=cc_in[KV * D:2 * KV * D, :].rearrange("(a p) n -> p a n",
                                                      p=128),
            in_=v_sb.rearrange("p (a n) -> p a n", n=KV * D))
        nc.gpsimd.collective_compute(
            "AllGather", ALU.bypass, replica_groups=GROUPS,
            ins=[cc_in.opt()], outs=[cc_out.opt()])
        ctx_kv.close()

        # ================= Q projection (overlaps collective) ============
        qT_sb = qT_pool.tile([128, H * SC], bf16)   # per head: [d, 512 t]
        for oc in range(OC):
            if oc == 0:
                wq_sb = wq_sb0
            else:
                wq_sb = w_pool.tile([128, IT * 512], bf16, tag="w",
                                    name="wq_sb")
                nc.sync.dma_start(out=wq_sb[:], in_=wqP[oc])
            ps_qs = []
            ssum_q = scr_pool.tile([128, TT * 4], f32, tag="ssum",
                                   name="ssum_q")
            for tt in range(TT):
                ps_q = mm_ps.tile([128, 512], f32, tag="mm", name="ps_q")
                for it in range(IT):
                    nc.tensor.matmul(ps_q[:], hs_tile(it, tt),
                                     wq_sb[:, it * 512:(it + 1) * 512],
                                     start=(it == 0), stop=(it == IT - 1))
                squares(ps_q, 4, ssum_q, tt * 4)
                ps_qs.append(ps_q)
            scq = scales_of(ssum_q, TT * 4)
            qT_v = qT_sb.rearrange("p (h t) -> p h t", t=SC)
            for tt in range(TT):
                qrope = rope_pool.tile([128, 4 * D], bf16, tag="qrope",
                                       name="qrope")
                rope_apply(ps_qs[tt], 4, scq[:, tt * 4:(tt + 1) * 4],
                           cwq_sb[:, tt * D:(tt + 1) * D],
                           swq_sb[:, tt * D:(tt + 1) * D], qrope)
                pstq = tr_ps.tile([128, 4 * 128], bf16, tag="tr", name="pstq")
                for hh in range(4):
                    nc.tensor.transpose(pstq[:, hh * 128:(hh + 1) * 128],
                                        qrope[:, hh * D:(hh + 1) * D],
                                        ident[:])
                nc.vector.tensor_copy(
                    qT_v[:, oc * 4:(oc + 1) * 4, tt * 128:(tt + 1) * 128],
                    pstq.rearrange("p (h t) -> p h t", t=128))
        ctx_hs.close()

        # ================= load gathered K/V =================
        # cc_out rank rr block of 1024 rows: [0:512] kT (g*128+d, s_local),
        #                                    [512:1024] v (s_local, g*128+d)
        kT_big = kv_pool.tile([128, KV * S], bf16, tag="kT")    # [d,(g rr s)]
        v_big = kv_pool.tile([128, ST * KV * D], bf16, tag="v")  # [s,(st hd)]
        kT_bv = kT_big.rearrange("p (g r s) -> p g r s", r=TPG, s=SC)
        for rr in range(TPG):
            base = rr * 2 * KV * D
            nc.sync.dma_start(
                out=kT_bv[:, :, rr, :],
                in_=cc_out[base: base + KV * D, :].rearrange(
                    "(g p) s -> p g s", p=128))
            nc.sync.dma_start(
                out=v_big[:,
                          rr * TT * KV * D:(rr + 1) * TT * KV * D].rearrange(
                    "p (a n) -> p a n", n=KV * D),
                in_=cc_out[base + KV * D: base + 2 * KV * D, :].rearrange(
                    "(a p) n -> p a n", p=128))

        def kT_tile(g, st):
            # stationary [128 d, 128 s] for s-tile st (st = rr*4 + sub)
            rr, sub = st // TT, st % TT
            off = (g * TPG + rr) * SC + sub * 128
            return kT_big[:, off: off + 128]

        def v_tile(g, st):
            # stationary [128 s, 128 d] for s-tile st
            off = st * KV * D + g * D
            return v_big[:, off: off + D]

        # ================= attention =================
        p2_pool = ep(tc.tile_pool(name="p2", bufs=2))
        attnT_pool = ep(tc.tile_pool(name="attnT", bufs=1))
        den_pool = ep(tc.tile_pool(name="den", bufs=1))
        sc_ps = ep(tc.tile_pool(name="sc_ps", bufs=2, space="PSUM"))
        att_ps = ep(tc.tile_pool(name="att_ps", bufs=4, space="PSUM"))

        attnT_sb = attnT_pool.tile([128, H * SC], bf16)
        for pr in range(H // 2):
            ha = 2 * pr
            g = ha // (H // KV)
            # p2 layout: [128, (st 16, h 2, n 512)] -> exp writes contiguous
            p2 = p2_pool.tile([128, ST * 2 * SC], bf16, tag="p2", name="p2")
            ps_att = [att_ps.tile([128, SC], f32, tag="att", name="ps_att")
                      for _ in range(2)]

            def av(st):
                for hh in range(2):
                    nc.tensor.matmul(
                        ps_att[hh][:], v_tile(g, st),
                        p2[:, st * 1024 + hh * 512:
                            st * 1024 + (hh + 1) * 512],
                        start=(st == 0), stop=(st == ST - 1))

            # AV matmuls lag the score matmuls by 2 s-tiles so the PE always
            # has runnable work while exp paces the score psum rotation --
            # keeps the scalar engine (the attention bottleneck) saturated.
            for st in range(ST):
                sc_t = sc_ps.tile([128, 1024], f32, tag="sc", name="sc_t")
                nc.tensor.matmul(sc_t[:, 0:512], kT_tile(g, st),
                                 qT_sb[:, ha * SC:(ha + 1) * SC],
                                 start=True, stop=True)
                nc.tensor.matmul(sc_t[:, 512:1024], kT_tile(g, st),
                                 qT_sb[:, (ha + 1) * SC:(ha + 2) * SC],
                                 start=True, stop=True)
                nc.scalar.activation(p2[:, st * 1024:(st + 1) * 1024],
                                     sc_t[:], AF.Exp)
                if st >= 2:
                    av(st - 2)
            av(ST - 2)
            av(ST - 1)
            p2v = p2.rearrange("p (a h n) -> p h a n", h=2, n=SC)
            for hh in range(2):
                h = ha + hh
                # denominator: tree-add the 16 exp tiles on DVE, then one
                # ones-matmul for the partition sum (broadcast for free)
                acc = den_pool.tile([128, 4 * SC], bf16, tag="acc",
                                    name="acc")
                nc.vector.tensor_add(acc[:], p2v[:, hh, 0:4, :],
                                     p2v[:, hh, 4:8, :])
                nc.vector.tensor_add(acc[:], acc[:], p2v[:, hh, 8:12, :])
                nc.vector.tensor_add(acc[:], acc[:], p2v[:, hh, 12:16, :])
                t2b = den_pool.tile([128, 2 * SC], bf16, tag="t2b",
                                    name="t2b")
                nc.vector.tensor_add(t2b[:], acc[:, 0:2 * SC],
                                     acc[:, 2 * SC:4 * SC])
                den_b = den_pool.tile([128, SC], bf16, tag="denb",
                                      name="den_b")
                nc.vector.tensor_add(den_b[:], t2b[:, 0:SC], t2b[:, SC:2 * SC])
                ps_db = att_ps.tile([128, SC], f32, tag="att", name="ps_db")
                nc.tensor.matmul(ps_db[:], ones_bf[:], den_b[:],
                                 start=True, stop=True)
                rden = osb_pool.tile([128, SC], f32, tag="osb", name="rden")
                nc.vector.reciprocal_approx_fast(rden[:], ps_db[:])
                nc.vector.tensor_mul(attnT_sb[:, h * SC:(h + 1) * SC],
                                     ps_att[hh][:], rden[:])

        # ================= output projection =================
        for oc in range(OC):
            wo_sb = w_pool.tile([128, IT * 512], bf16, tag="w", name="wo_sb")
            nc.sync.dma_start(out=wo_sb[:], in_=woP[oc])
            for fl in range(4):
                ps_o = att_ps.tile([128, SC], f32, tag="att", name="ps_o")
                for kk in range(IT):
                    nc.tensor.matmul(
                        ps_o[:],
                        wo_sb[:, kk * 512 + fl * 128: kk * 512 + (fl + 1) * 128],
                        attnT_sb[:, kk * SC:(kk + 1) * SC],
                        start=(kk == 0), stop=(kk == IT - 1))
                ft = oc * 4 + fl
                o_sb = osb_pool.tile([128, SC], f32, tag="osb", name="o_sb")
                nc.vector.tensor_copy(o_sb[:], ps_o[:])
                nc.sync.dma_start(out=outT[ft * 128:(ft + 1) * 128, :],
                                  in_=o_sb[:])
    finally:
        ctx.close()
        tc_cm.__exit__(None, None, None)

    nc.compile()
    return nc


def _prep_inputs(hidden_states, cos, sin, Wq, Wk, Wv, Wo, norm_q_w,
                 norm_k_w):
    """Host-side: transpose + bf16-cast weights into tile-major layouts,
    fold norm weights + 1/sqrt(D) into rope coefficients, slice per core."""
    import ml_dtypes
    f = np.float32
    bf = ml_dtypes.bfloat16
    hs = np.asarray(hidden_states, f)
    cos = np.asarray(cos, f)
    sin = np.asarray(sin, f)

    def tile_major(wT, oc_split):
        # wT: [HS, N] -> [oc][128, it, 512] (tile-major over rows)
        n = wT.shape[1]
        arr = wT.reshape(IT, 128, n)
        if oc_split:
            out = np.empty((OC, 128, IT * 512), bf)
            for oc in range(OC):
                blk = arr[:, :, oc * 512:(oc + 1) * 512]  # [it, 128, 512]
                out[oc] = blk.transpose(1, 0, 2).reshape(128, IT * 512)
            return out
        return np.ascontiguousarray(
            arr.transpose(1, 0, 2).reshape(128, IT * 512)).astype(bf)

    wq = tile_major(np.asarray(Wq, f).T, True)       # [4, 128, 8192]
    wk = tile_major(np.asarray(Wk, f).T, False)      # [128, 8192]
    wv = tile_major(np.asarray(Wv, f).T, False)
    wo = tile_major(np.asarray(Wo, f).T, True)
    wqn = np.asarray(norm_q_w, f)
    wkn = np.asarray(norm_k_w, f)

    def rope_consts(w, scale):
        # cw[t, d] = cos[t, d] * w[d] * scale
        # sw[t, d<64]  = -sin[t, d] * w[d+64] * scale
        # sw[t, d>=64] = +sin[t, d] * w[d-64] * scale
        cw = cos * w[None, :] * scale
        w_swap = np.concatenate([w[D // 2:], w[:D // 2]])
        sgn = np.concatenate([-np.ones(D // 2, f), np.ones(D // 2, f)])
        sw = sin * (w_swap * sgn)[None, :] * scale
        return cw.astype(f), sw.astype(f)

    cwq_full, swq_full = rope_consts(wqn, np.float32(D ** -0.5))
    cwk_full, swk_full = rope_consts(wkn, np.float32(1.0))

    def part_major(a):
        # [512, D] -> [128, tt, D] -> [128, tt*D]
        return np.ascontiguousarray(
            a.reshape(TT, 128, D).transpose(1, 0, 2).reshape(128, TT * D))

    in_maps = []
    for c in range(NCORES):
        b, j = divmod(c, TPG)
        sl = slice(j * SC, (j + 1) * SC)
        hsT = hs[b].T[:, sl]                          # [2048 i, 512 t]
        hsp = hsT.reshape(IT, 128, SC).transpose(1, 0, 2).reshape(
            128, IT * SC).astype(bf)
        in_maps.append({
            "hsP": np.ascontiguousarray(hsp),
            "cwq": part_major(cwq_full[sl]),
            "swq": part_major(swq_full[sl]),
            "cwk": part_major(cwk_full[sl]),
            "swk": part_major(swk_full[sl]),
            "wqP": wq, "wkP": wk, "wvP": wv, "woP": wo,
        })
    return in_maps


def _assemble(results):
    out = np.empty((B, S, HS), np.float32)
    for c in range(NCORES):
        b, j = divmod(c, TPG)
        out[b, j * SC:(j + 1) * SC, :] = results[c]["outT"].T
    return out


def kernel(hidden_states, cos, sin, Wq, Wk, Wv, Wo, norm_q_w, norm_k_w,
           _run_kwargs=None):
    from concourse.bass_utils import run_bass_kernel_spmd

    if "nc" not in _BUILT:
        _BUILT["nc"] = _build_program()
    nc = _BUILT["nc"]
    in_maps = _prep_inputs(hidden_states, cos, sin, Wq, Wk, Wv, Wo,
                           norm_q_w, norm_k_w)
    kw = _run_kwargs or {}
    res = run_bass_kernel_spmd(nc, in_maps, list(range(NCORES)), **kw)
    _BUILT["last_results"] = res
    return _assemble(res.results)
